# revision 17
# baseline (speedup 1.0000x reference)
import os
import sys

import numpy as np
import ml_dtypes

for _p in ("/opt/trn_rl_repo",):
    if _p not in sys.path and os.path.isdir(_p):
        sys.path.insert(0, _p)

B, S, D = 2, 2048, 1024
H = 16
DH = 64
DVH = 64
HPC = 4
NCORES = 8
NEG = -1.0e9
SCALE = 1.0 / 8.0

BF16 = ml_dtypes.bfloat16

_CACHE = {}


def _build_program(repeat=1):
    import concourse.bass as bass
    import concourse.tile as tile
    from concourse import bacc, mybir

    dt = mybir.dt
    nc = bacc.Bacc("TRN2", target_bir_lowering=False, debug=False,
                   num_devices=NCORES)

    xt_d = nc.dram_tensor("xt", [8, 128, S], dt.bfloat16, kind="ExternalInput").ap()
    wq_d = nc.dram_tensor("wq", [8, 128, 256], dt.bfloat16, kind="ExternalInput").ap()
    wk_d = nc.dram_tensor("wk", [8, 128, 256], dt.bfloat16, kind="ExternalInput").ap()
    wv_d = nc.dram_tensor("wv", [8, 128, 256], dt.bfloat16, kind="ExternalInput").ap()
    bq_d = nc.dram_tensor("bq", [128, 2], dt.float32, kind="ExternalInput").ap()
    bk_d = nc.dram_tensor("bk", [128, 2], dt.float32, kind="ExternalInput").ap()
    am_d = nc.dram_tensor("amask", [128, 128], dt.float32, kind="ExternalInput").ap()
    out_d = nc.dram_tensor("out", [HPC, 65, S], dt.float32, kind="ExternalOutput").ap()

    for _ in range(repeat):
        _build_body(nc, tile, mybir,
                    xt_d, wq_d, wk_d, wv_d, bq_d, bk_d, am_d, out_d)

    nc.compile()
    return nc


def _build_body(nc, tile, mybir, xt_d, wq_d, wk_d, wv_d, bq_d, bk_d, am_d, out_d):
    dt = mybir.dt
    Exp = mybir.ActivationFunctionType.Exp

    with tile.TileContext(nc) as tc:
        with (
            tc.tile_pool(name="const", bufs=1) as const,
            tc.tile_pool(name="expp", bufs=4) as expp,
            tc.tile_pool(name="osb", bufs=2) as osb,
        ):
            xt_sb = const.tile([128, 8, S], dt.bfloat16)
            wq_sb = const.tile([128, 8, 256], dt.bfloat16)
            wk_sb = const.tile([128, 8, 256], dt.bfloat16)
            wv_sb = const.tile([128, 8, 256], dt.bfloat16)
            bq_sb = const.tile([128, 2], dt.float32)
            bk_sb = const.tile([128, 2], dt.float32)
            am_sb = const.tile([128, 128], dt.float32)
            qt_sb = const.tile([128, 2, S], dt.bfloat16)
            kt_sb = const.tile([128, 2, S], dt.bfloat16)
            v_sb = const.tile([128, 16, HPC, 65], dt.bfloat16)

            nc.sync.dma_start(wq_sb[:, :, :], wq_d.rearrange("c p n -> p c n"))
            nc.scalar.dma_start(wk_sb[:, :, :], wk_d.rearrange("c p n -> p c n"))
            rings = [nc.sync, nc.scalar]
            for ct in (0, 1):
                for kc in range(8):
                    rings[kc % 2].dma_start(
                        xt_sb[:, kc, 512 * ct:512 * (ct + 1)],
                        xt_d[kc, :, 512 * ct:512 * (ct + 1)])
            nc.sync.dma_start(bq_sb[:, :], bq_d)
            nc.scalar.dma_start(bk_sb[:, :], bk_d)
            nc.sync.dma_start(am_sb[:, :], am_d)
            nc.scalar.dma_start(wv_sb[:, :, :], wv_d.rearrange("c p n -> p c n"))
            for ct in (2, 3):
                for kc in range(8):
                    rings[kc % 2].dma_start(
                        xt_sb[:, kc, 512 * ct:512 * (ct + 1)],
                        xt_d[kc, :, 512 * ct:512 * (ct + 1)])

            QH = 1024

            def proj_qk_ct(pool, tag, p, ct, which):
                dst_sb, w_sb, b_sb = ((qt_sb, wq_sb, bq_sb),
                                      (kt_sb, wk_sb, bk_sb))[which]
                ps = pool.tile([128, 512], dt.float32, tag=tag, name="ps")
                for kc in range(8):
                    nc.tensor.matmul(
                        ps,
                        w_sb[:, kc, 128 * p:128 * (p + 1)],
                        xt_sb[:, kc, 512 * ct:512 * (ct + 1)],
                        start=(kc == 0), stop=(kc == 7),
                    )
                nc.vector.tensor_scalar_add(
                    dst_sb[:, p, 512 * ct:512 * (ct + 1)], ps, b_sb[:, p:p + 1])

            def proj_v(pool, tag, sc_lo, sc_hi):
                for sc in range(sc_lo, sc_hi):
                    ps2 = pool.tile([128, HPC, 64], dt.float32, tag=tag, name="ps2")
                    for kc in range(8):
                        nc.tensor.matmul(
                            ps2,
                            xt_sb[:, kc, 128 * sc:128 * (sc + 1)],
                            wv_sb[:, kc, :],
                            start=(kc == 0), stop=(kc == 7),
                        )
                    nc.vector.tensor_copy(v_sb[:, sc, :, 0:64], ps2)

            nc.vector.memset(v_sb[:, :, :, 64], 1.0)
            with tc.tile_pool(name="pps", bufs=4, space="PSUM") as pps:
                for ct in (0, 1):
                    proj_qk_ct(pps, "pp", 0, ct, 0)
                for ct in (0, 1):
                    proj_qk_ct(pps, "pp", 0, ct, 1)

            stp = tc.alloc_tile_pool(name="stp", bufs=3, space="PSUM")
            op = tc.alloc_tile_pool(name="op", bufs=1, space="PSUM")

            def attn_head(h, halves=(0, 1)):
                p, hi = h // 2, h % 2
                base = 64 * hi
                for qh in halves:
                    h0, h1 = QH * qh, QH * (qh + 1)
                    outp = op.tile([65, QH], dt.float32, tag="op", name="outp")
                    for kb in range(16):
                        cq0 = max(128 * kb, h0)
                        if cq0 >= h1:
                            continue
                        clen = h1 - cq0
                        isdiag = 128 * kb >= h0
                        st = stp.tile([128, clen], dt.float32, tag="st", name="st")
                        n0 = 0
                        while n0 < clen:
                            nl = min(512, clen - n0)
                            nc.tensor.matmul(
                                st[:, n0:n0 + nl],
                                kt_sb[base:base + 64, p, 128 * kb:128 * kb + 128],
                                qt_sb[base:base + 64, p, cq0 + n0:cq0 + n0 + nl],
                                start=True, stop=True,
                            )
                            n0 += nl
                        if isdiag:
                            nc.vector.tensor_add(st[:, 0:128], st[:, 0:128], am_sb)
                        ext = expp.tile([128, clen], dt.bfloat16, tag="ex", name="ext")
                        nc.scalar.activation(ext, st, Exp, scale=SCALE)
                        segs = []
                        s0 = cq0
                        if isdiag:
                            segs.append((cq0, 128, True))
                            s0 = cq0 + 128
                        while s0 < h1:
                            s1 = min((s0 // 512 + 1) * 512, h1)
                            segs.append((s0, s1 - s0, False))
                            s0 = s1
                        for (g0, gl, isd) in segs:
                            nc.tensor.matmul(
                                outp[:, g0 - h0:g0 - h0 + gl],
                                v_sb[:, kb, h, :],
                                ext[:, g0 - cq0:g0 - cq0 + gl],
                                start=(kb == 0 and g0 % 512 == 0),
                                stop=(isd and kb % 4 == 3),
                            )
                    ot = osb.tile([65, QH], dt.float32, tag="ot", name="ot")
                    for oc in range(2):
                        nc.vector.tensor_copy(ot[:, 512 * oc:512 * (oc + 1)],
                                              outp[:, 512 * oc:512 * (oc + 1)])
                    nc.sync.dma_start(out_d[h, :, h0:h1], ot)

            proj_v(stp, "st", 0, 8)
            attn_head(0, halves=(0,))
            for ct in (2, 3):
                proj_qk_ct(stp, "st", 0, ct, 0)
                proj_qk_ct(stp, "st", 0, ct, 1)
            proj_v(stp, "st", 8, 16)
            attn_head(0, halves=(1,))
            for ct in range(4):
                proj_qk_ct(stp, "st", 1, ct, 0)
                proj_qk_ct(stp, "st", 1, ct, 1)
            attn_head(1)
            attn_head(2)
            attn_head(3)
            op.release()
            stp.release()


def _get_program():
    if "nc" not in _CACHE:
        _CACHE["nc"] = _build_program()
    return _CACHE["nc"]


def make_in_maps(x, Wqk, bqk, Wv, bv):
    ii, jj = np.meshgrid(np.arange(128), np.arange(128), indexing="ij")
    amask = np.where(ii <= jj, 0.0, NEG).astype(np.float32)
    in_maps = []
    for c in range(NCORES):
        b, g = divmod(c, 4)
        cols = slice(256 * g, 256 * (g + 1))
        xt = np.ascontiguousarray(x[b].T).astype(BF16).reshape(8, 128, S)
        wq = np.ascontiguousarray(Wqk[:, cols]).astype(BF16).reshape(8, 128, 256)
        wk = np.ascontiguousarray(Wqk[:, D:][:, cols]).astype(BF16).reshape(8, 128, 256)
        wv = np.ascontiguousarray(Wv[:, cols]).astype(BF16).reshape(8, 128, 256)
        bq = np.ascontiguousarray(bqk[cols].reshape(2, 128).T).astype(np.float32)
        bk = np.ascontiguousarray(bqk[D:][cols].reshape(2, 128).T).astype(np.float32)
        in_maps.append({"xt": xt, "wq": wq, "wk": wk, "wv": wv,
                        "bq": bq, "bk": bk, "amask": amask})
    return in_maps


def assemble(per_core_out, bv):
    out = np.empty((B, S, H * DVH), np.float32)
    for c in range(NCORES):
        b, g = divmod(c, 4)
        o = per_core_out[c]
        for hh in range(HPC):
            hg = HPC * g + hh
            a = o[hh, :64, :] / o[hh, 64:65, :]
            out[b, :, DVH * hg:DVH * (hg + 1)] = a.T + bv[DVH * hg:DVH * (hg + 1)]
    return out


def kernel(x, Wqk, bqk, Wv, bv):
    from concourse.bass_utils import run_bass_kernel_spmd

    nc = _get_program()
    in_maps = make_in_maps(np.asarray(x, np.float32), np.asarray(Wqk, np.float32),
                           np.asarray(bqk, np.float32), np.asarray(Wv, np.float32),
                           np.asarray(bv, np.float32))
    trace = os.environ.get("MHA_TRACE", "0") == "1"
    res = run_bass_kernel_spmd(nc, in_maps, list(range(NCORES)), trace=trace)
    _CACHE["last_result"] = res
    return assemble([r["out"] for r in res.results], np.asarray(bv, np.float32))


# revision 25
# speedup vs baseline: 1.0039x; 1.0039x over previous
import os
import sys

import numpy as np
import ml_dtypes

for _p in ("/opt/trn_rl_repo",):
    if _p not in sys.path and os.path.isdir(_p):
        sys.path.insert(0, _p)

B, S, D = 2, 2048, 1024
H = 16
DH = 64
DVH = 64
HPC = 4
NCORES = 8
SCALE = 1.0 / 8.0
QH = 1024

BF16 = ml_dtypes.bfloat16

_CACHE = {}


def _build_program(repeat=1):
    import concourse.tile as tile
    from concourse import bacc, mybir

    dt = mybir.dt
    nc = bacc.Bacc("TRN2", target_bir_lowering=False, debug=False,
                   num_devices=NCORES)

    xt_d = nc.dram_tensor("xt", [8, 128, S], dt.bfloat16, kind="ExternalInput").ap()
    wq_d = nc.dram_tensor("wq", [8, 128, 256], dt.bfloat16, kind="ExternalInput").ap()
    wk_d = nc.dram_tensor("wk", [8, 128, 256], dt.bfloat16, kind="ExternalInput").ap()
    wv_d = nc.dram_tensor("wv", [8, 128, 256], dt.bfloat16, kind="ExternalInput").ap()
    bq_d = nc.dram_tensor("bq", [128, 2], dt.float32, kind="ExternalInput").ap()
    bk_d = nc.dram_tensor("bk", [128, 2], dt.float32, kind="ExternalInput").ap()
    am_d = nc.dram_tensor("amask", [128, 128], dt.bfloat16, kind="ExternalInput").ap()
    out_d = nc.dram_tensor("out", [HPC, 65, S], dt.float32, kind="ExternalOutput").ap()

    for _ in range(repeat):
        _build_body(nc, tile, mybir,
                    xt_d, wq_d, wk_d, wv_d, bq_d, bk_d, am_d, out_d)

    nc.compile()
    return nc


def _build_body(nc, tile, mybir, xt_d, wq_d, wk_d, wv_d, bq_d, bk_d, am_d, out_d):
    dt = mybir.dt
    Exp = mybir.ActivationFunctionType.Exp

    with tile.TileContext(nc) as tc:
        with (
            tc.tile_pool(name="const", bufs=1) as const,
            tc.tile_pool(name="expp", bufs=8) as expp,
            tc.tile_pool(name="osb", bufs=2) as osb,
        ):
            xt_sb = const.tile([128, 8, S], dt.bfloat16)
            wq_sb = const.tile([128, 8, 256], dt.bfloat16)
            wk_sb = const.tile([128, 8, 256], dt.bfloat16)
            wv_sb = const.tile([128, 8, 256], dt.bfloat16)
            bq_sb = const.tile([128, 2], dt.float32)
            bk_sb = const.tile([128, 2], dt.float32)
            am_sb = const.tile([128, 128], dt.bfloat16)
            qt_sb = const.tile([128, 2, S], dt.bfloat16)
            kt_sb = const.tile([128, 2, S], dt.bfloat16)
            v_sb = const.tile([128, 16, HPC, 65], dt.bfloat16)

            nc.sync.dma_start(wq_sb[:, :, :], wq_d.rearrange("c p n -> p c n"))
            nc.scalar.dma_start(wk_sb[:, :, :], wk_d.rearrange("c p n -> p c n"))
            rings = [nc.sync, nc.scalar]
            for ct in (0, 1):
                for kc in range(8):
                    rings[kc % 2].dma_start(
                        xt_sb[:, kc, 512 * ct:512 * (ct + 1)],
                        xt_d[kc, :, 512 * ct:512 * (ct + 1)])
            nc.sync.dma_start(bq_sb[:, :], bq_d)
            nc.scalar.dma_start(bk_sb[:, :], bk_d)
            nc.sync.dma_start(am_sb[:, :], am_d)
            nc.scalar.dma_start(wv_sb[:, :, :], wv_d.rearrange("c p n -> p c n"))
            for ct in (2, 3):
                for kc in range(8):
                    rings[kc % 2].dma_start(
                        xt_sb[:, kc, 512 * ct:512 * (ct + 1)],
                        xt_d[kc, :, 512 * ct:512 * (ct + 1)])

            def proj_qk_ct(pool, tag, p, ct, which):
                dst_sb, w_sb, b_sb = ((qt_sb, wq_sb, bq_sb),
                                      (kt_sb, wk_sb, bk_sb))[which]
                ps = pool.tile([128, 512], dt.float32, tag=tag, name="ps")
                for kc in range(8):
                    nc.tensor.matmul(
                        ps,
                        w_sb[:, kc, 128 * p:128 * (p + 1)],
                        xt_sb[:, kc, 512 * ct:512 * (ct + 1)],
                        start=(kc == 0), stop=(kc == 7),
                    )
                nc.vector.tensor_scalar_add(
                    dst_sb[:, p, 512 * ct:512 * (ct + 1)], ps, b_sb[:, p:p + 1])

            def proj_v(pool, tag, sc_lo, sc_hi):
                for sc in range(sc_lo, sc_hi):
                    ps2 = pool.tile([128, HPC, 64], dt.float32, tag=tag, name="ps2")
                    for kc in range(8):
                        nc.tensor.matmul(
                            ps2,
                            xt_sb[:, kc, 128 * sc:128 * (sc + 1)],
                            wv_sb[:, kc, :],
                            start=(kc == 0), stop=(kc == 7),
                        )
                    nc.vector.tensor_copy(v_sb[:, sc, :, 0:64], ps2)

            nc.vector.memset(v_sb[:, :, :, 64], 1.0)
            stp = tc.alloc_tile_pool(name="stp", bufs=2, space="PSUM")
            op = tc.alloc_tile_pool(name="op", bufs=1, space="PSUM")
            pp = tc.alloc_tile_pool(name="pp", bufs=2, space="PSUM")

            def attn_head(h, halves=(0, 1)):
                p, hi = h // 2, h % 2
                base = 64 * hi
                for qh in halves:
                    h0, h1 = QH * qh, QH * (qh + 1)
                    outp = op.tile([65, QH], dt.float32, tag="op", name="outp")
                    for kb in range(16):
                        cq0 = max(128 * kb, h0)
                        if cq0 >= h1:
                            continue
                        clen = h1 - cq0
                        isdiag = 128 * kb >= h0
                        st = stp.tile([128, clen], dt.float32, tag="st", name="st")
                        n0 = 0
                        while n0 < clen:
                            nl = min(512, clen - n0)
                            nc.tensor.matmul(
                                st[:, n0:n0 + nl],
                                kt_sb[base:base + 64, p, 128 * kb:128 * kb + 128],
                                qt_sb[base:base + 64, p, cq0 + n0:cq0 + n0 + nl],
                                start=True, stop=True,
                            )
                            n0 += nl
                        ext = expp.tile([128, clen], dt.bfloat16, tag="ex", name="ext")
                        nc.scalar.activation(ext, st, Exp, scale=SCALE)
                        if isdiag:
                            nc.vector.tensor_mul(ext[:, 0:128], ext[:, 0:128],
                                                 am_sb)
                        segs = []
                        s0 = cq0
                        if isdiag:
                            segs.append((cq0, 128, True))
                            s0 = cq0 + 128
                        while s0 < h1:
                            s1 = min((s0 // 512 + 1) * 512, h1)
                            segs.append((s0, s1 - s0, False))
                            s0 = s1
                        for (g0, gl, isd) in segs:
                            nc.tensor.matmul(
                                outp[:, g0 - h0:g0 - h0 + gl],
                                v_sb[:, kb, h, :],
                                ext[:, g0 - cq0:g0 - cq0 + gl],
                                start=(kb == 0 and g0 % 512 == 0),
                                stop=(isd and kb % 4 == 3),
                            )
                    ot = osb.tile([65, QH], dt.float32, tag="ot", name="ot")
                    for oc in range(2):
                        nc.vector.tensor_copy(ot[:, 512 * oc:512 * (oc + 1)],
                                              outp[:, 512 * oc:512 * (oc + 1)])
                    nc.sync.dma_start(out_d[h, :, h0:h1], ot)

            for ct in (0, 1):
                proj_qk_ct(pp, "pp", 0, ct, 0)
                proj_qk_ct(pp, "pp", 0, ct, 1)
            proj_v(pp, "pp", 0, 8)
            attn_head(0, halves=(0,))
            proj_v(pp, "pp", 8, 16)
            for ct in (2, 3):
                proj_qk_ct(pp, "pp", 0, ct, 0)
                proj_qk_ct(pp, "pp", 0, ct, 1)
            attn_head(0, halves=(1,))
            attn_head(1)
            for ct in range(4):
                proj_qk_ct(pp, "pp", 1, ct, 0)
                proj_qk_ct(pp, "pp", 1, ct, 1)
            attn_head(2)
            attn_head(3)
            pp.release()
            op.release()
            stp.release()


def _get_program():
    if "nc" not in _CACHE:
        _CACHE["nc"] = _build_program()
    return _CACHE["nc"]


def make_in_maps(x, Wqk, bqk, Wv, bv):
    ii, jj = np.meshgrid(np.arange(128), np.arange(128), indexing="ij")
    amask = np.where(ii <= jj, 1.0, 0.0).astype(BF16)
    in_maps = []
    for c in range(NCORES):
        b, g = divmod(c, 4)
        cols = slice(256 * g, 256 * (g + 1))
        xt = np.ascontiguousarray(x[b].T).astype(BF16).reshape(8, 128, S)
        wq = np.ascontiguousarray(Wqk[:, cols]).astype(BF16).reshape(8, 128, 256)
        wk = np.ascontiguousarray(Wqk[:, D:][:, cols]).astype(BF16).reshape(8, 128, 256)
        wv = np.ascontiguousarray(Wv[:, cols]).astype(BF16).reshape(8, 128, 256)
        bq = np.ascontiguousarray(bqk[cols].reshape(2, 128).T).astype(np.float32)
        bk = np.ascontiguousarray(bqk[D:][cols].reshape(2, 128).T).astype(np.float32)
        in_maps.append({"xt": xt, "wq": wq, "wk": wk, "wv": wv,
                        "bq": bq, "bk": bk, "amask": amask})
    return in_maps


def assemble(per_core_out, bv):
    out = np.empty((B, S, H * DVH), np.float32)
    for c in range(NCORES):
        b, g = divmod(c, 4)
        o = per_core_out[c]
        for hh in range(HPC):
            hg = HPC * g + hh
            a = o[hh, :64, :] / o[hh, 64:65, :]
            out[b, :, DVH * hg:DVH * (hg + 1)] = a.T + bv[DVH * hg:DVH * (hg + 1)]
    return out


def kernel(x, Wqk, bqk, Wv, bv):
    from concourse.bass_utils import run_bass_kernel_spmd

    nc = _get_program()
    in_maps = make_in_maps(np.asarray(x, np.float32), np.asarray(Wqk, np.float32),
                           np.asarray(bqk, np.float32), np.asarray(Wv, np.float32),
                           np.asarray(bv, np.float32))
    trace = os.environ.get("MHA_TRACE", "0") == "1"
    res = run_bass_kernel_spmd(nc, in_maps, list(range(NCORES)), trace=trace)
    _CACHE["last_result"] = res
    return assemble([r["out"] for r in res.results], np.asarray(bv, np.float32))


# revision 28
# speedup vs baseline: 1.7098x; 1.7032x over previous
import os
import sys

import numpy as np
import ml_dtypes

for _p in ("/opt/trn_rl_repo",):
    if _p not in sys.path and os.path.isdir(_p):
        sys.path.insert(0, _p)

B, S, D = 2, 2048, 1024
H = 16
DH = 64
DVH = 64
HPC = 4
NCORES = 8
SCALE = 1.0 / 8.0
QH = 512

BF16 = ml_dtypes.bfloat16

_CACHE = {}


def _build_program(repeat=1):
    import concourse.tile as tile
    from concourse import bacc, mybir

    dt = mybir.dt
    nc = bacc.Bacc("TRN2", target_bir_lowering=False, debug=False,
                   num_devices=NCORES)

    xt_d = nc.dram_tensor("xt", [8, 128, S], dt.bfloat16, kind="ExternalInput").ap()
    wq_d = nc.dram_tensor("wq", [8, 128, 256], dt.bfloat16, kind="ExternalInput").ap()
    wk_d = nc.dram_tensor("wk", [8, 128, 256], dt.bfloat16, kind="ExternalInput").ap()
    wv_d = nc.dram_tensor("wv", [8, 128, 256], dt.bfloat16, kind="ExternalInput").ap()
    bq_d = nc.dram_tensor("bq", [128, 2], dt.float32, kind="ExternalInput").ap()
    bk_d = nc.dram_tensor("bk", [128, 2], dt.float32, kind="ExternalInput").ap()
    am_d = nc.dram_tensor("amask", [128, 128], dt.bfloat16, kind="ExternalInput").ap()
    out_d = nc.dram_tensor("out", [HPC, 65, S], dt.float32, kind="ExternalOutput").ap()

    for _ in range(repeat):
        _build_body(nc, tile, mybir,
                    xt_d, wq_d, wk_d, wv_d, bq_d, bk_d, am_d, out_d)

    nc.compile()
    return nc


def _build_body(nc, tile, mybir, xt_d, wq_d, wk_d, wv_d, bq_d, bk_d, am_d, out_d):
    dt = mybir.dt
    Exp = mybir.ActivationFunctionType.Exp

    with tile.TileContext(nc) as tc:
        with (
            tc.tile_pool(name="const", bufs=1) as const,
            tc.tile_pool(name="expp", bufs=8) as expp,
            tc.tile_pool(name="osb", bufs=2) as osb,
        ):
            xt_sb = const.tile([128, 8, S], dt.bfloat16)
            wq_sb = const.tile([128, 8, 256], dt.bfloat16)
            wk_sb = const.tile([128, 8, 256], dt.bfloat16)
            wv_sb = const.tile([128, 8, 256], dt.bfloat16)
            bq_sb = const.tile([128, 2], dt.float32)
            bk_sb = const.tile([128, 2], dt.float32)
            am_sb = const.tile([128, 128], dt.bfloat16)
            qt_sb = const.tile([128, 2, S], dt.bfloat16)
            kt_sb = const.tile([128, 2, S], dt.bfloat16)
            v_sb = const.tile([128, 16, HPC, 65], dt.bfloat16)

            nc.sync.dma_start(wq_sb[:, :, :], wq_d.rearrange("c p n -> p c n"))
            nc.scalar.dma_start(wk_sb[:, :, :], wk_d.rearrange("c p n -> p c n"))
            rings = [nc.sync, nc.scalar]
            for ct in (0, 1):
                for kc in range(8):
                    rings[kc % 2].dma_start(
                        xt_sb[:, kc, 512 * ct:512 * (ct + 1)],
                        xt_d[kc, :, 512 * ct:512 * (ct + 1)])
            nc.sync.dma_start(bq_sb[:, :], bq_d)
            nc.scalar.dma_start(bk_sb[:, :], bk_d)
            nc.sync.dma_start(am_sb[:, :], am_d)
            nc.scalar.dma_start(wv_sb[:, :, :], wv_d.rearrange("c p n -> p c n"))
            for ct in (2, 3):
                for kc in range(8):
                    rings[kc % 2].dma_start(
                        xt_sb[:, kc, 512 * ct:512 * (ct + 1)],
                        xt_d[kc, :, 512 * ct:512 * (ct + 1)])

            def proj_qk_ct(pool, tag, p, ct, which):
                dst_sb, w_sb, b_sb = ((qt_sb, wq_sb, bq_sb),
                                      (kt_sb, wk_sb, bk_sb))[which]
                ps = pool.tile([128, 512], dt.float32, tag=tag, name="ps")
                for kc in range(8):
                    nc.tensor.matmul(
                        ps,
                        w_sb[:, kc, 128 * p:128 * (p + 1)],
                        xt_sb[:, kc, 512 * ct:512 * (ct + 1)],
                        start=(kc == 0), stop=(kc == 7),
                    )
                nc.vector.tensor_scalar_add(
                    dst_sb[:, p, 512 * ct:512 * (ct + 1)], ps, b_sb[:, p:p + 1])

            def proj_v(pool, tag, sc_lo, sc_hi):
                for sc in range(sc_lo, sc_hi):
                    ps2 = pool.tile([128, HPC, 64], dt.float32, tag=tag, name="ps2")
                    for kc in range(8):
                        nc.tensor.matmul(
                            ps2,
                            xt_sb[:, kc, 128 * sc:128 * (sc + 1)],
                            wv_sb[:, kc, :],
                            start=(kc == 0), stop=(kc == 7),
                        )
                    nc.vector.tensor_copy(v_sb[:, sc, :, 0:64], ps2)

            nc.vector.memset(v_sb[:, :, :, 64], 1.0)
            stp = tc.alloc_tile_pool(name="stp", bufs=5, space="PSUM")
            op = tc.alloc_tile_pool(name="op", bufs=2, space="PSUM")
            pp = tc.alloc_tile_pool(name="pp", bufs=1, space="PSUM")

            from collections import deque
            pend = deque()
            cur_out = {}
            LAG = 3

            def emit_av_one():
                (u, h, h0, h1, kb, cq0, clen, isdiag, ext, is_last) = pend.popleft()
                if u not in cur_out:
                    cur_out[u] = op.tile([65, QH], dt.float32, tag="op",
                                         name="outp")
                outp = cur_out[u]
                segs = []
                s0 = cq0
                if isdiag:
                    segs.append((cq0, 128, True))
                    s0 = cq0 + 128
                while s0 < h1:
                    s1 = min((s0 // 512 + 1) * 512, h1)
                    segs.append((s0, s1 - s0, False))
                    s0 = s1
                for (g0, gl, isd) in segs:
                    nc.tensor.matmul(
                        outp[:, g0 - h0:g0 - h0 + gl],
                        v_sb[:, kb, h, :],
                        ext[:, g0 - cq0:g0 - cq0 + gl],
                        start=(kb == 0 and g0 % 512 == 0),
                        stop=(isd and kb % 4 == 3),
                    )
                if is_last:
                    h_, h0_ = h, h0
                    ot = osb.tile([65, QH], dt.float32, tag="ot", name="ot")
                    nc.vector.tensor_copy(ot, outp)
                    nc.sync.dma_start(out_d[h_, :, h0_:h0_ + QH], ot)
                    del cur_out[u]

            def attn_head(h, halves=(0, 1, 2, 3)):
                p, hi = h // 2, h % 2
                base = 64 * hi
                for qh in halves:
                    h0, h1 = QH * qh, QH * (qh + 1)
                    u = (h, qh)
                    kbs = [kb for kb in range(16) if max(128 * kb, h0) < h1]
                    for kb in kbs:
                        cq0 = max(128 * kb, h0)
                        clen = h1 - cq0
                        isdiag = 128 * kb >= h0
                        st = stp.tile([128, clen], dt.float32, tag="st", name="st")
                        n0 = 0
                        while n0 < clen:
                            nl = min(512, clen - n0)
                            nc.tensor.matmul(
                                st[:, n0:n0 + nl],
                                kt_sb[base:base + 64, p, 128 * kb:128 * kb + 128],
                                qt_sb[base:base + 64, p, cq0 + n0:cq0 + n0 + nl],
                                start=True, stop=True,
                            )
                            n0 += nl
                        ext = expp.tile([128, clen], dt.bfloat16, tag="ex", name="ext")
                        nc.scalar.activation(ext, st, Exp, scale=SCALE)
                        if isdiag:
                            nc.vector.tensor_mul(ext[:, 0:128], ext[:, 0:128],
                                                 am_sb)
                        pend.append((u, h, h0, h1, kb, cq0, clen, isdiag, ext,
                                     kb == kbs[-1]))
                        while len(pend) > LAG:
                            emit_av_one()

            def attn_drain():
                while pend:
                    emit_av_one()

            for ct in (0, 1):
                proj_qk_ct(pp, "pp", 0, ct, 0)
                proj_qk_ct(pp, "pp", 0, ct, 1)
            proj_v(pp, "pp", 0, 8)
            attn_head(0, halves=(0, 1))
            proj_v(pp, "pp", 8, 16)
            for ct in (2, 3):
                proj_qk_ct(pp, "pp", 0, ct, 0)
                proj_qk_ct(pp, "pp", 0, ct, 1)
            attn_head(0, halves=(2, 3))
            attn_head(1)
            for ct in range(4):
                proj_qk_ct(pp, "pp", 1, ct, 0)
                proj_qk_ct(pp, "pp", 1, ct, 1)
            attn_head(2)
            attn_head(3)
            attn_drain()
            pp.release()
            op.release()
            stp.release()


def _get_program():
    if "nc" not in _CACHE:
        _CACHE["nc"] = _build_program()
    return _CACHE["nc"]


def make_in_maps(x, Wqk, bqk, Wv, bv):
    ii, jj = np.meshgrid(np.arange(128), np.arange(128), indexing="ij")
    amask = np.where(ii <= jj, 1.0, 0.0).astype(BF16)
    in_maps = []
    for c in range(NCORES):
        b, g = divmod(c, 4)
        cols = slice(256 * g, 256 * (g + 1))
        xt = np.ascontiguousarray(x[b].T).astype(BF16).reshape(8, 128, S)
        wq = np.ascontiguousarray(Wqk[:, cols]).astype(BF16).reshape(8, 128, 256)
        wk = np.ascontiguousarray(Wqk[:, D:][:, cols]).astype(BF16).reshape(8, 128, 256)
        wv = np.ascontiguousarray(Wv[:, cols]).astype(BF16).reshape(8, 128, 256)
        bq = np.ascontiguousarray(bqk[cols].reshape(2, 128).T).astype(np.float32)
        bk = np.ascontiguousarray(bqk[D:][cols].reshape(2, 128).T).astype(np.float32)
        in_maps.append({"xt": xt, "wq": wq, "wk": wk, "wv": wv,
                        "bq": bq, "bk": bk, "amask": amask})
    return in_maps


def assemble(per_core_out, bv):
    out = np.empty((B, S, H * DVH), np.float32)
    for c in range(NCORES):
        b, g = divmod(c, 4)
        o = per_core_out[c]
        for hh in range(HPC):
            hg = HPC * g + hh
            a = o[hh, :64, :] / o[hh, 64:65, :]
            out[b, :, DVH * hg:DVH * (hg + 1)] = a.T + bv[DVH * hg:DVH * (hg + 1)]
    return out


def kernel(x, Wqk, bqk, Wv, bv):
    from concourse.bass_utils import run_bass_kernel_spmd

    nc = _get_program()
    in_maps = make_in_maps(np.asarray(x, np.float32), np.asarray(Wqk, np.float32),
                           np.asarray(bqk, np.float32), np.asarray(Wv, np.float32),
                           np.asarray(bv, np.float32))
    trace = os.environ.get("MHA_TRACE", "0") == "1"
    try:
        res = run_bass_kernel_spmd(nc, in_maps, list(range(NCORES)), trace=trace)
    except Exception:
        if not trace:
            raise
        res = run_bass_kernel_spmd(nc, in_maps, list(range(NCORES)), trace=False)
    _CACHE["last_result"] = res
    return assemble([r["out"] for r in res.results], np.asarray(bv, np.float32))
